# revision 21
# baseline (speedup 1.0000x reference)
"""MoE gating kernel for Trainium2 (8 NeuronCores, SPMD).

Computes, for x [4, 4096, 2048] f32 and W [64, 2048] f32:
    logits = x_flat @ W.T          # [16384, 64]
    top2 values/indices (sorted descending), softmax over the top-2 logits
Returns (indices int32 [16384, 2], values f32 [16384, 2]) — matching
jax.lax.top_k + softmax in the reference.

Strategy (per core, 2048 tokens), all fp32 so indices agree exactly with
the fp32 reference:
  - DMA x naturally (tokens on partitions, D contiguous) — full HBM BW.
  - PE-transpose 128x128 tiles to put D on partitions (~109ns/tile issue
    rate, LDW-bound; transpose-mode ignores the HAM clock gate).
  - Gating matmul fp32 col-packed: even d-chunks accumulate into PSUM
    partitions 0:64, odd chunks into 64:128; W.T [128d, 64e] stationary,
    x.T [128d, Nt] moving.
  - Per block: transpose phase -> previous block's logits tail (PE part)
    -> matmul phase. PSUM->SBUF transpose copies all run on DVE (only
    engine that keeps up); the tail's DVE burst is emitted after the
    copies so it drains during the matmul phase when DVE is idle.
  - A dummy-matmul burst on the identity tile at kernel start flips the
    PE HAM clock gate to 8/8 while the first x DMA is in flight; real
    matmuls otherwise run at 1.2 GHz for the first ~30us.
  - ident/WT/output DMAs ride the gpsimd SWDGE queue so x loads lead the
    sync HWDGE FIFO; small blocks at both ends for fast fill/drain.
"""

import sys

for _p in ("/opt/trn_rl_repo", "/root/problem/work"):
    if _p not in sys.path:
        sys.path.insert(0, _p)

import numpy as np

import concourse.bass as bass
import concourse.mybir as mybir
from concourse.tile import TileContext
from concourse.bass_utils import run_bass_kernel_spmd

N_CORES = 8
TOKENS = 16384
D = 2048
E = 64
TOK_PER_CORE = TOKENS // N_CORES  # 2048
KCHUNKS = D // 128                # 16
NTILES = TOK_PER_CORE // 128      # 16
WARMUP_MMS = 28
BLOCK_TILES = [1, 2, 4, 4, 4, 1]  # token tiles per block (sums to 16)

F32 = mybir.dt.float32
U32 = mybir.dt.uint32
SIG = mybir.ActivationFunctionType.Sigmoid

_CACHE = {}


def _split_multi_waits(nc, max_waits=1):
    """walrus in this env supports only ONE sync wait per instruction's
    sync_info; split extras onto preceding NOPs on the same engine."""
    n = 0
    for fn in nc.m.functions:
        for bb in fn.blocks:
            out = []
            for inst in bb.instructions:
                si = inst.sync_info
                if si is not None and si.on_wait is not None and len(si.on_wait) > max_waits:
                    waits = list(si.on_wait)
                    head, tail = waits[:-max_waits], waits[-max_waits:]
                    k = 0
                    while head:
                        chunk, head = head[:max_waits], head[max_waits:]
                        out.append(mybir.InstNoOp(
                            name=f"{inst.name}-wsplit{k}",
                            engine=inst.engine, ins=[], outs=[],
                            sync_info=mybir.SyncInfo(on_wait=chunk, on_update=[]),
                        ))
                        k += 1
                        n += 1
                    inst.sync_info = mybir.SyncInfo(
                        on_wait=tail, on_update=list(si.on_update or []))
                out.append(inst)
            bb.instructions = out
    return n


def build_nc():
    nc = bass.Bass(trn_type="TRN2")
    x = nc.dram_tensor("x", [TOK_PER_CORE, D], F32, kind="ExternalInput")
    # W.T pre-laid-out host-side: wt[p, c*64+e] = W[e, c*128+p]
    wt_in = nc.dram_tensor("WT", [128, KCHUNKS * E], F32, kind="ExternalInput")
    ident_in = nc.dram_tensor("ident", [128, 128], F32, kind="ExternalInput")
    out_val = nc.dram_tensor("out_val", [128, NTILES, 2], F32, kind="ExternalOutput")
    out_idx = nc.dram_tensor("out_idx", [128, NTILES, 2], U32, kind="ExternalOutput")

    # token tile t (128 tokens each): token = t*128 + p -> [t, p, d]
    x_t = x.rearrange("(t p) d -> t p d", t=NTILES, p=128)

    with TileContext(nc) as tc:
        with (
            tc.tile_pool(name="singles", bufs=1) as singles,
            tc.tile_pool(name="xb", bufs=3) as xb_pool,
            tc.tile_pool(name="xt", bufs=24) as xt_pool,
            tc.tile_pool(name="lg", bufs=3) as lg_pool,
            tc.tile_pool(name="small", bufs=1) as small,
            tc.tile_pool(name="psT", bufs=5, space="PSUM") as psT_pool,
            tc.tile_pool(name="psL", bufs=1, space="PSUM") as psL_pool,
            tc.tile_pool(name="psS", bufs=2, space="PSUM") as psS_pool,
        ):
            ident = singles.tile([128, 128], F32)
            nc.sync.dma_start(out=ident, in_=ident_in[:, :])

            xb0 = xb_pool.tile([128, BLOCK_TILES[0], D], F32, tag="xb")
            nc.sync.dma_start(out=xb0, in_=x_t[0:BLOCK_TILES[0]]
                              .rearrange("t p d -> p t d"))
            wt = singles.tile([128, KCHUNKS * E], F32)
            nc.sync.dma_start(out=wt, in_=wt_in[:, :])

            # dense dummy-matmul burst on an uninitialized SBUF tile: no
            # DMA dependency, so it runs from t~0 and flips HAM to 8/8
            # while the DMA subsystem is still starting up (~8.5us dead)
            garb = singles.tile([128, 128], F32)
            nc.vector.memset(garb, 1.0)
            warm = psL_pool.tile([E, 512], F32, tag="psL")
            for _ in range(WARMUP_MMS):
                nc.tensor.matmul(warm[:, 0:128], lhsT=garb[:, 0:E],
                                 rhs=garb, start=True, stop=True)

            ltok = singles.tile([128, NTILES * E], F32)
            maxb = small.tile([128, NTILES, 8], F32)
            idxb = small.tile([128, NTILES, 8], U32)
            d10 = small.tile([128, NTILES], F32)
            valb = small.tile([128, NTILES, 2], F32)
            idxo = small.tile([128, NTILES, 2], U32)

            def make_tail(lgs, tiles):
                """Deferred per-block tail: logits re-transpose, half-sum,
                top-2, 2-way softmax, output DMA."""
                def tail():
                    for k, t in enumerate(tiles):
                        ps2 = psS_pool.tile([128, 128], F32, tag="psS")
                        nc.tensor.transpose(ps2, lgs[:, k * 128:(k + 1) * 128],
                                            ident)
                        lh = lg_pool.tile([128, E], F32, tag="lh")
                        nc.scalar.copy(out=lh, in_=ps2[:, E:128])
                        nc.vector.tensor_add(ltok[:, t * E:(t + 1) * E],
                                             ps2[:, 0:E], lh)
                    for t in tiles:
                        nc.vector.max(out=maxb[:, t, :],
                                      in_=ltok[:, t * E:(t + 1) * E])
                        nc.vector.max_index(out=idxb[:, t, :],
                                            in_max=maxb[:, t, :],
                                            in_values=ltok[:, t * E:(t + 1) * E])
                    s = slice(tiles[0], tiles[-1] + 1)
                    nc.vector.tensor_sub(d10[:, s], maxb[:, s, 1], maxb[:, s, 0])
                    nc.scalar.activation(valb[:, s, 1], d10[:, s], SIG)
                    nc.scalar.activation(valb[:, s, 0], d10[:, s], SIG,
                                         scale=-1.0)
                    nc.vector.tensor_copy(idxo[:, s, :], idxb[:, s, 0:2])
                    nc.sync.dma_start(out=out_val[:, s, :], in_=valb[:, s, :])
                    nc.sync.dma_start(out=out_idx[:, s, :], in_=idxo[:, s, :])
                return tail

            ci = [0]

            def do_block(t0, jtiles, xb, pending_tail):
                """One block: transpose phase (mixed DVE/ACT copies, the
                previous block's tail emitted mid-stream), matmul phase,
                ACT logits copy. Returns deferred tail."""
                width = jtiles * 128
                xts = []
                for c in range(KCHUNKS):
                    psT = psT_pool.tile([128, width], F32, tag="psT")
                    for j in range(jtiles):
                        nc.tensor.transpose(
                            psT[:, j * 128:(j + 1) * 128],
                            xb[:, j, c * 128:(c + 1) * 128],
                            ident)
                    xt = xt_pool.tile([128, width], F32, tag="xt")
                    if (ci[0] % 16) < 16:    # DVE copies ~2x faster than ACT
                        nc.vector.tensor_copy(xt, psT)
                    else:
                        nc.scalar.copy(out=xt, in_=psT)
                    ci[0] += 1
                    xts.append(xt)
                    if c == 3 and pending_tail is not None:
                        pending_tail()
                psl = psL_pool.tile([128, width], F32, tag="psL")
                for c in range(KCHUNKS):
                    g = c % 2
                    nc.tensor.matmul(
                        psl[g * E:(g + 1) * E, :],
                        lhsT=wt[:, c * E:(c + 1) * E], rhs=xts[c],
                        start=(c < 2), stop=(c >= KCHUNKS - 2))
                lgs = lg_pool.tile([128, width], F32, tag="lg")
                nc.scalar.copy(out=lgs, in_=psl)
                return make_tail(lgs, list(range(t0, t0 + jtiles)))

            t0 = 0
            pending = None
            for bi, jt in enumerate(BLOCK_TILES):
                if bi == 0:
                    xb = xb0
                else:
                    xb = xb_pool.tile([128, jt, D], F32, tag="xb")
                    nc.sync.dma_start(out=xb, in_=x_t[t0:t0 + jt]
                                      .rearrange("t p d -> p t d"))
                pending = do_block(t0, jt, xb, pending)
                t0 += jt
            pending()  # last block's tail

    _split_multi_waits(nc)
    return nc


def _get_nc():
    if "nc" not in _CACHE:
        _CACHE["nc"] = build_nc()
    return _CACHE["nc"]


def kernel(x: np.ndarray, W: np.ndarray, _trace=False, _tmpdir=None):
    nc = _get_nc()
    x_flat = np.ascontiguousarray(x.reshape(TOKENS, D).astype(np.float32))
    Wc = W.astype(np.float32)
    wt = np.ascontiguousarray(
        Wc.reshape(E, KCHUNKS, 128).transpose(2, 1, 0).reshape(128, KCHUNKS * E))
    ident = np.eye(128, dtype=np.float32)
    in_maps = [
        {"x": x_flat[c * TOK_PER_CORE:(c + 1) * TOK_PER_CORE], "WT": wt,
         "ident": ident}
        for c in range(N_CORES)
    ]
    res = run_bass_kernel_spmd(nc, in_maps, core_ids=list(range(N_CORES)),
                               trace=_trace, tmpdir=_tmpdir)
    _CACHE["last_result"] = res
    idx_parts, val_parts = [], []
    for c in range(N_CORES):
        r = res.results[c]
        # [128p, 16t, 2] -> token local = t*128 + p
        val_parts.append(r["out_val"].transpose(1, 0, 2).reshape(TOK_PER_CORE, 2))
        idx_parts.append(r["out_idx"].transpose(1, 0, 2).reshape(TOK_PER_CORE, 2)
                         .astype(np.int32))
    return (np.concatenate(idx_parts, 0), np.concatenate(val_parts, 0))
